# revision 11
# baseline (speedup 1.0000x reference)
"""Trainium2 Bass kernel for nn_Attention (Gaussian banded attention).

Math (reference):
    v = values @ input_weights.T                      # [B,L,D]
    probs[h,q,k] = N(k - q - off_h; std_h)            # Gaussian, depends on k-q only
    attended[b,h,q,:] = sum_k probs[h,q,k] v[b,k,h*pd:(h+1)*pd]
    out = attended_merged @ output_weight.T           # [B,L,D]

Key structural facts exploited:
  - probs is a banded Toeplitz matrix per head: nonzero only for
    k - q in [off - 6*std, off + 6*std] (6-sigma truncation, error ~1e-8).
    Widest band: std=8, off=-8 -> k-q in [-56, 40].
  - So attention is a narrow depthwise convolution along L; no [L,L] matmul.
  - Batch x L sharding is embarrassingly parallel given a halo of
    56 backward / 40 forward rows of the INPUT (v is a row-wise projection,
    zero rows project to zero since there is no bias).

Sharding: 8 cores = (B=2) x (4 chunks of 512 rows of L). Each core gets
x.T zero-padded to [1024, 640] (56 halo + 512 + 40 halo + 32 zero pad),
computes in [D, L]->[L, D]->[D, L] layouts on the TensorEngine in bf16,
and writes out.T [1024, 512] f32. Host reassembles. No collectives.
"""

import math
from contextlib import ExitStack

import numpy as np
import ml_dtypes

import concourse.bass as bass
from concourse import mybir
from concourse.bass_utils import run_bass_kernel_spmd

# ---------------- problem constants (hardcoded per spec) ----------------
B, L, D = 2, 2048, 1024
H, PD = 8, 128
ATTN_STD = np.array([1.0, 2.0, 4.0, 8.0, 1.0, 2.0, 4.0, 8.0], dtype=np.float64)
ATTN_OFFSET = np.array([-1.0, -2.0, -4.0, -8.0, -1.0, -2.0, -4.0, -8.0], dtype=np.float64)

N_CORES = 8
CHUNK = 512            # output rows per core
HALO_L, HALO_R = 56, 40
LPAD = 640             # 56 + 512 + 40 = 608, padded to 5*128
LT = 5                 # l-tiles of v (640 / 128)
KT = 8                 # d tiles (1024 / 128)
NQ = CHUNK             # query columns per core

BF16 = mybir.dt.bfloat16
F32 = mybir.dt.float32

G1 = LT * 2            # proj1 groups: (l-tile, n-chunk) -> v
G2 = H                 # attention heads -> attendedT
G3 = KT                # proj2 d_out tiles -> outT
NPS = 4                # rotating PSUM banks


def gauss_toeplitz_table() -> np.ndarray:
    """tp[h, r, m] = g_h(r - (m - 512) - 56), shape [H, 128, 1024] bf16.

    For v-tile t (rows k' = 128t + r of padded-local v) the attention rhs is
    tp[h][:, 512-128t : 1024-128t] so that rhs[r, q'] = g_h(128t + r - q' - 56),
    which is probs[h, q, k].T in padded-local coordinates.
    """
    r = np.arange(128, dtype=np.float64)[:, None]
    m = np.arange(1024, dtype=np.float64)[None, :]
    delta = r - (m - 512.0) - 56.0  # = k - q
    tables = []
    for h in range(H):
        std, off = ATTN_STD[h], ATTN_OFFSET[h]
        z = (delta - off) / std
        g = np.exp(-0.5 * z * z) / (std * math.sqrt(2.0 * math.pi))
        g[np.abs(z) > 6.0] = 0.0
        tables.append(g)
    return np.stack(tables).astype(ml_dtypes.bfloat16)


def attn_windows(h: int):
    """Static (t, j0, j1) list: nonzero q-column window of v-tile t for head h,
    8-aligned. Coverage of [0,512) is guaranteed (window width > 128)."""
    std, off = int(ATTN_STD[h]), int(ATTN_OFFSET[h])
    wlo = -56 - off - 6 * std
    whi = 71 - off + 6 * std
    res = []
    for t in range(LT):
        j0 = max(0, 128 * t + wlo)
        j1 = min(NQ, 128 * t + whi + 1)
        if j0 >= j1:
            continue
        j0 = (j0 // 8) * 8
        j1 = min(NQ, ((j1 + 7) // 8) * 8)
        res.append((t, j0, j1))
    return res


def build_graph(iters: int = 1, banded: bool = True) -> bass.Bass:
    """One SPMD core program. iters>1 repeats the whole kernel (including
    DMAs) with monotonically increasing semaphore thresholds, for timing."""
    nc = bass.Bass()

    xt = nc.declare_dram_parameter("xt", [D, LPAD], BF16, isOutput=False)
    w1t = nc.declare_dram_parameter("w1t", [D, D], BF16, isOutput=False)
    w2t = nc.declare_dram_parameter("w2t", [D, D], BF16, isOutput=False)
    tp = nc.declare_dram_parameter("tp", [H, 128, 1024], BF16, isOutput=False)
    out = nc.declare_dram_parameter("out", [D, NQ], F32, isOutput=True)

    xt_r = xt[:].rearrange("(o p) f -> p o f", p=128)    # [128, 8, 640]
    w1_r = w1t[:].rearrange("(o p) f -> p o f", p=128)   # [128, 8, 1024]
    w2_r = w2t[:].rearrange("(o p) f -> p o f", p=128)   # [128, 8, 1024]
    tp_r = tp[:].rearrange("h p f -> p h f")             # [128, 8, 1024]

    with ExitStack() as ctx:
        e = ctx.enter_context
        xt_sb = e(nc.sbuf_tensor("xt_sb", [128, KT, LPAD], BF16))
        w1_sb = e(nc.sbuf_tensor("w1_sb", [128, KT, D], BF16))
        w2_sb = e(nc.sbuf_tensor("w2_sb", [128, KT, D], BF16))
        TP0, TPW = (408, 240) if banded else (0, 1024)
        tp_sb = e(nc.sbuf_tensor("tp_sb", [128, H, TPW], BF16))
        tp_src = tp_r[:, :, TP0:TP0 + TPW]
        v_sb = e(nc.sbuf_tensor("v_sb", [128, LT, D], BF16))
        at_sb = e(nc.sbuf_tensor("at_sb", [128, H, NQ], BF16))
        o_sb = e(nc.sbuf_tensor("o_sb", [128, KT, NQ], F32))
        ps = [e(nc.psum_tensor(f"ps{i}", [128, 512], F32)) for i in range(8)]

        sem_names = (["mm1", "mm2", "mm3", "tp_d",
                      "cp1v", "cp1s", "cp2v", "cp2s", "cp3v", "cp3s"]
                     + [f"xt_d{k}" for k in range(KT)]
                     + [f"w1_d{k}" for k in range(KT)]
                     + [f"w2_d{k}" for k in range(KT)]
                     + [f"dmo{m}" for m in range(G3)])
        sems = {n: e(nc.semaphore(n)) for n in sem_names}

        # copies alternate engines: even group -> vector, odd group -> scalar
        def ncop(g_total, parity):
            return len([g for g in range(g_total) if g % 2 == parity])

        PER_PHASE = {1: (ncop(G1, 0), ncop(G1, 1)),
                     2: (ncop(G2, 0), ncop(G2, 1)),
                     3: (ncop(G3, 0), ncop(G3, 1))}

        def cp_sem_for(phase, g):
            return sems[f"cp{phase}{'v' if g % 2 == 0 else 's'}"]

        def cp_count(phase, g, it):
            """That engine's copy-sem value after group g of iteration it."""
            n = PER_PHASE[phase][g % 2]
            return it * n + g // 2 + 1

        with nc.Block() as block:

            @block.sync
            def _(sync: bass.BassEngine):
                for it in range(iters):
                    if it > 0:
                        # WAR: xt/w1 re-read finished once prev phase 1 is done
                        sync.wait_ge(sems["mm1"], it * G1)
                    for k in range(KT):
                        sync.dma_start(out=xt_sb[:, k, :], in_=xt_r[:, k, :]).then_inc(
                            sems[f"xt_d{k}"], 16)
                        sync.dma_start(out=w1_sb[:, k, :], in_=w1_r[:, k, :]).then_inc(
                            sems[f"w1_d{k}"], 16)
                    if it == 0:
                        sync.dma_start(out=tp_sb[:], in_=tp_src).then_inc(
                            sems["tp_d"], 16)
                    if it > 0:
                        sync.wait_ge(sems["mm3"], it * G3)
                    for k in range(KT):
                        sync.dma_start(out=w2_sb[:, k, :], in_=w2_r[:, k, :]).then_inc(
                            sems[f"w2_d{k}"], 16)

            @block.tensor
            def _(tensor: bass.BassEngine):
                for it in range(iters):
                    # ---- phase 1: v[l, d] = x @ W_in.T  (banks 0-3) ----
                    for g in range(G1):
                        lt, n = g // 2, g % 2
                        bank = ps[g % NPS]
                        if g < NPS:
                            if it > 0:
                                # bank freed by prev iter's phase-3 copy (m = bank+4)
                                m_prev = g % NPS + 4
                                tensor.wait_ge(cp_sem_for(3, m_prev),
                                               cp_count(3, m_prev, it - 1))
                        else:
                            tensor.wait_ge(cp_sem_for(1, g - NPS),
                                           cp_count(1, g - NPS, it))
                        for k in range(KT):
                            if g == 0:
                                tensor.wait_ge(sems[f"xt_d{k}"], (it + 1) * 16)
                                tensor.wait_ge(sems[f"w1_d{k}"], (it + 1) * 16)
                            mm = tensor.matmul(
                                bank[:, :],
                                xt_sb[:, k, 128 * lt:128 * lt + 128],
                                w1_sb[:, k, 512 * n:512 * n + 512],
                                start=(k == 0), stop=(k == KT - 1),
                            )
                            if k == KT - 1:
                                mm.then_inc(sems["mm1"])

                    # ---- phase 2: attendedT[pd, q] per head (banks 4-7) ----
                    if it == 0:
                        tensor.wait_ge(sems["tp_d"], 16)
                    for h in range(G2):
                        bank = ps[4 + h % NPS]
                        # bank WAR
                        if h < NPS:
                            if it > 0:
                                tensor.wait_ge(cp_sem_for(2, h + 4),
                                               cp_count(2, h + 4, it - 1))
                        else:
                            tensor.wait_ge(cp_sem_for(2, h - NPS),
                                           cp_count(2, h - NPS, it))
                        windows = attn_windows(h) if banded else [
                            (t, 0, NQ) for t in range(LT)]
                        for wi, (t, j0, j1) in enumerate(windows):
                            # data: window t reads v tile t cols of n-chunk h//4,
                            # produced by phase-1 group g1 = 2t + h//4.
                            g1 = 2 * t + h // 4
                            tensor.wait_ge(cp_sem_for(1, g1), cp_count(1, g1, it))
                            c0 = 512 - 128 * t + j0 - TP0
                            c1 = 512 - 128 * t + j1 - TP0
                            mm = tensor.matmul(
                                bank[:, j0:j1],
                                v_sb[:, t, 128 * h:128 * h + 128],
                                tp_sb[:, h, c0:c1],
                                start=(wi == 0), stop=(wi == len(windows) - 1),
                            )
                            if wi == len(windows) - 1:
                                mm.then_inc(sems["mm2"])

                    # ---- phase 3: outT[d_out, l] = W_out @ attendedT (banks 0-3) ----
                    for m in range(G3):
                        bank = ps[m % NPS]
                        # bank WAR: last phase-1 user of this bank copied out
                        b = m % NPS
                        g_last = b + 8 if b < 2 else b + 4
                        tensor.wait_ge(cp_sem_for(1, g_last),
                                       cp_count(1, g_last, it))
                        if m >= NPS:
                            tensor.wait_ge(cp_sem_for(3, m - NPS),
                                           cp_count(3, m - NPS, it))
                        for k in range(KT):
                            if m == 0:
                                # data: head k's attendedT copy done
                                tensor.wait_ge(cp_sem_for(2, k),
                                               cp_count(2, k, it))
                                tensor.wait_ge(sems[f"w2_d{k}"], (it + 1) * 16)
                            mm = tensor.matmul(
                                bank[:, :],
                                w2_sb[:, k, 128 * m:128 * m + 128],
                                at_sb[:, k, :],
                                start=(k == 0), stop=(k == KT - 1),
                            )
                            if k == KT - 1:
                                mm.then_inc(sems["mm3"])

            def copier(engine, parity):
                def copy_op(out, in_):
                    if hasattr(engine, "tensor_copy"):
                        return engine.tensor_copy(out=out, in_=in_)
                    return engine.copy(out, in_)

                for it in range(iters):
                    for g in range(G1):
                        if g % 2 != parity:
                            continue
                        lt, n = g // 2, g % 2
                        engine.wait_ge(sems["mm1"], it * G1 + g + 1)
                        copy_op(v_sb[:, lt, 512 * n:512 * n + 512],
                                ps[g % NPS][:, :]).then_inc(cp_sem_for(1, g))
                    for h in range(G2):
                        if h % 2 != parity:
                            continue
                        engine.wait_ge(sems["mm2"], it * G2 + h + 1)
                        copy_op(at_sb[:, h, :],
                                ps[4 + h % NPS][:, :]).then_inc(cp_sem_for(2, h))
                    for m in range(G3):
                        if m % 2 != parity:
                            continue
                        engine.wait_ge(sems["mm3"], it * G3 + m + 1)
                        if it > 0:
                            # WAR: previous iteration's output DMA of this m done
                            engine.wait_ge(sems[f"dmo{m}"], it * 16)
                        copy_op(o_sb[:, m, :],
                                ps[m % NPS][:, :]).then_inc(cp_sem_for(3, m))

            @block.vector
            def _(vector: bass.BassEngine):
                copier(vector, 0)

            @block.scalar
            def _(scalar: bass.BassEngine):
                copier(scalar, 1)

            @block.gpsimd
            def _(gpsimd: bass.BassEngine):
                for it in range(iters):
                    for m in range(G3):
                        gpsimd.wait_ge(cp_sem_for(3, m), cp_count(3, m, it))
                        gpsimd.dma_start(
                            out=out[128 * m:128 * m + 128, :],
                            in_=o_sb[:, m, :],
                        ).then_inc(sems[f"dmo{m}"], 16)
                for m in range(G3):
                    gpsimd.wait_ge(sems[f"dmo{m}"], iters * 16)

    return nc


# ---------------- host side ----------------

_GRAPH_CACHE: dict = {}


def get_graph(iters: int = 1, banded: bool = True) -> bass.Bass:
    key = (iters, banded)
    if key not in _GRAPH_CACHE:
        _GRAPH_CACHE[key] = build_graph(iters, banded)
    return _GRAPH_CACHE[key]


class Runner:
    """Compile-once executor for one Bass graph across the 8 cores.

    Mirrors bass2jax.run_bass_via_pjrt but keeps the jitted callable so
    repeated invocations don't re-trace/re-compile.
    """

    def __init__(self, nc: bass.Bass, n_cores: int = N_CORES):
        import jax
        from jax.sharding import Mesh, PartitionSpec
        from jax.experimental.shard_map import shard_map
        from concourse import bass2jax, mybir as _mb

        bass2jax.install_neuronx_cc_hook()
        self.n_cores = n_cores

        partition_name = (nc.partition_id_tensor.name
                          if nc.partition_id_tensor else None)
        in_names, out_names, out_avals, zero_shapes = [], [], [], []
        for alloc in nc.m.functions[0].allocations:
            if not isinstance(alloc, _mb.MemoryLocationSet):
                continue
            name = alloc.memorylocations[0].name
            if alloc.kind == "ExternalInput":
                if name != partition_name:
                    in_names.append(name)
            elif alloc.kind == "ExternalOutput":
                out_names.append(name)
                shape = tuple(alloc.tensor_shape)
                dtype = _mb.dt.np(alloc.dtype)
                out_avals.append(jax.core.ShapedArray(shape, dtype))
                zero_shapes.append((shape, dtype))
        self.in_names = list(in_names)
        self.out_names = out_names
        self.out_avals = out_avals
        self.zero_shapes = zero_shapes
        n_params = len(in_names)
        all_names = in_names + out_names
        if partition_name is not None:
            all_names = all_names + [partition_name]

        def _body(*args):
            operands = list(args)
            if partition_name is not None:
                operands.append(bass2jax.partition_id_tensor())
            outs = bass2jax._bass_exec_p.bind(
                *operands,
                out_avals=tuple(out_avals),
                in_names=tuple(all_names),
                out_names=tuple(out_names),
                lowering_input_output_aliases=(),
                sim_require_finite=True,
                sim_require_nnan=True,
                nc=nc,
            )
            return tuple(outs)

        devices = jax.devices()[:n_cores]
        mesh = Mesh(np.asarray(devices), ("core",))
        n_outs = len(out_names)
        self._fn = jax.jit(
            shard_map(_body, mesh=mesh,
                      in_specs=(PartitionSpec("core"),) * (n_params + n_outs),
                      out_specs=(PartitionSpec("core"),) * n_outs,
                      check_rep=False),
            donate_argnums=tuple(range(n_params, n_params + n_outs)),
            keep_unused=True,
        )

    def __call__(self, in_maps):
        concat_in = [
            np.concatenate([np.asarray(m[name]) for m in in_maps], axis=0)
            for name in self.in_names
        ]
        zeros = [np.zeros((self.n_cores * s[0], *s[1:]), d)
                 for s, d in self.zero_shapes]
        out_arrs = self._fn(*concat_in, *zeros)
        return [
            {name: np.asarray(out_arrs[i]).reshape(
                self.n_cores, *self.out_avals[i].shape)[c]
             for i, name in enumerate(self.out_names)}
            for c in range(self.n_cores)
        ]


_RUNNER_CACHE: dict = {}


def get_runner(iters: int = 1) -> "Runner":
    if iters not in _RUNNER_CACHE:
        _RUNNER_CACHE[iters] = Runner(get_graph(iters))
    return _RUNNER_CACHE[iters]


def make_in_maps(values: np.ndarray, input_weights: np.ndarray,
                 output_weight: np.ndarray) -> list:
    bf = ml_dtypes.bfloat16
    w1t = np.ascontiguousarray(input_weights.T).astype(bf)
    w2t = np.ascontiguousarray(output_weight.T).astype(bf)
    tpt = gauss_toeplitz_table()
    in_maps = []
    for core in range(N_CORES):
        b, c = divmod(core, 4)
        lo, hi = c * CHUNK - HALO_L, c * CHUNK + CHUNK + HALO_R
        src_lo, src_hi = max(lo, 0), min(hi, L)
        xt_pad = np.zeros((D, LPAD), dtype=bf)
        xt_pad[:, src_lo - lo:src_hi - lo] = values[b, src_lo:src_hi, :].T.astype(bf)
        in_maps.append({"xt": xt_pad, "w1t": w1t, "w2t": w2t, "tp": tpt})
    return in_maps


def assemble(results: list) -> np.ndarray:
    out = np.empty((B, L, D), dtype=np.float32)
    for core in range(N_CORES):
        b, c = divmod(core, 4)
        out[b, c * CHUNK:(c + 1) * CHUNK, :] = results[core]["out"].T
    return out


def kernel(values: np.ndarray, input_weights: np.ndarray,
           output_weight: np.ndarray) -> np.ndarray:
    runner = get_runner(1)
    in_maps = make_in_maps(values, input_weights, output_weight)
    return assemble(runner(in_maps))
